# revision 1
# baseline (speedup 1.0000x reference)
"""Sparse avg-pool (segment mean) for Trainium2, 8 NeuronCores — TensorEngine version.

Range-shard coarse ids across cores (core k owns ids [k*31360, (k+1)*31360)),
so no collective is needed.  On each core the segment-sum runs on the
TensorEngine: the host sorts the core's rows by local id and buckets them into
245 windows of 128 consecutive ids, padding each window to `cap` tiles of 128
tokens.  For each 128-token tile the DVE builds a one-hot [token, seg] matrix
(is_equal of the token's window-relative id against an iota row), and the PE
accumulates onehot^T @ [feats | 1] into a per-window [128, 66] PSUM bank in
f32 (bf16 multiplicands: features round once to bf16, counts are exact).  A
DVE epilogue divides sums by max(count, 1) and DMAs the window's 128 output
rows.  No GPSIMD scatter ucode anywhere — the old dma_scatter_add version was
descriptor-generation bound at ~7 ns/token.
"""
import os
import sys
from dataclasses import dataclass

sys.path.insert(0, "/opt/trn_rl_repo")

import numpy as np

NCORES = 8
C = 64
CW = 66  # 64 feats + count + window-relative id
W = 128  # ids per window


@dataclass(frozen=True)
class Cfg:
    n_coarse_pad: int = 250_880  # 8 * 245 * 128
    cap: int = 9                 # tiles of 128 tokens per window
    load_windows: int = 8        # windows per input DMA

    @property
    def rng(self):
        return self.n_coarse_pad // NCORES

    @property
    def n_win(self):  # windows per core
        return self.rng // W

    @property
    def s_slots(self):  # 128-token slots per core
        return self.n_win * self.cap

    @property
    def s_tot(self):
        return self.s_slots * 128


FUSED_OH = bool(int(os.environ.get("KERNEL_FUSED_OH", "1")))

CFG = Cfg()
_nc_cache = {}
LAST_RESULT = None


def build_nc(cfg: Cfg):
    from concourse import bacc, mybir, tile

    bf16 = mybir.dt.bfloat16
    f32 = mybir.dt.float32
    nc = bacc.Bacc("TRN2", target_bir_lowering=False)
    feats_ext = nc.declare_dram_parameter(
        "feats", [128, cfg.s_slots, CW], bf16, isOutput=False
    )
    iota_ext = nc.declare_dram_parameter("iota", [128, W], bf16, isOutput=False)
    out_ext = nc.declare_dram_parameter(
        "out", [cfg.n_win, W, C], f32, isOutput=True
    )

    lw = cfg.load_windows
    n_chunks = (cfg.n_win + lw - 1) // lw
    assert cfg.n_win % lw == 0 or True

    with tile.TileContext(nc) as tc:
        with (
            tc.tile_pool(name="stage", bufs=2) as stagep,
            tc.tile_pool(name="oh", bufs=4) as ohp,
            tc.tile_pool(name="psum", bufs=8, space="PSUM") as psump,
            tc.tile_pool(name="fin", bufs=4) as finp,
            tc.tile_pool(name="cst", bufs=1) as cstp,
        ):
            iota_t = cstp.tile([128, W], bf16)
            nc.sync.dma_start(out=iota_t[:], in_=iota_ext[:])

            for ch in range(n_chunks):
                w0 = ch * lw
                nw = min(lw, cfg.n_win - w0)
                src = stagep.tile([128, lw * cfg.cap, CW], bf16, tag="src")
                nc.sync.dma_start(
                    out=src[:, : nw * cfg.cap, :],
                    in_=feats_ext[:, w0 * cfg.cap : (w0 + nw) * cfg.cap, :],
                )
                for wi in range(nw):
                    w = w0 + wi
                    ps = psump.tile([128, CW], f32, tag="ps")
                    if FUSED_OH:
                        s0 = wi * cfg.cap
                        ohw = ohp.tile([128, cfg.cap, W], bf16, tag="ohw")
                        nc.vector.tensor_tensor(
                            out=ohw[:],
                            in0=src[:, s0 : s0 + cfg.cap, CW - 1 : CW].to_broadcast(
                                [128, cfg.cap, W]
                            ),
                            in1=iota_t[:].unsqueeze(1).to_broadcast(
                                [128, cfg.cap, W]
                            ),
                            op=mybir.AluOpType.is_equal,
                        )
                    for j in range(cfg.cap):
                        s = wi * cfg.cap + j
                        if FUSED_OH:
                            oh = ohw[:, j, :]
                        else:
                            oht = ohp.tile([128, W], bf16, tag="oh")
                            nc.vector.tensor_tensor(
                                out=oht[:],
                                in0=src[:, s, CW - 1 : CW].to_broadcast([128, W]),
                                in1=iota_t[:],
                                op=mybir.AluOpType.is_equal,
                            )
                            oh = oht[:]
                        nc.tensor.matmul(
                            out=ps[:],
                            lhsT=oh,
                            rhs=src[:, s, :CW],
                            start=(j == 0),
                            stop=(j == cfg.cap - 1),
                        )
                    den = finp.tile([128, 1], f32, tag="den")
                    nc.vector.tensor_scalar_max(den[:], ps[:, C : C + 1], 1.0)
                    inv = finp.tile([128, 1], f32, tag="inv")
                    nc.vector.reciprocal(inv[:], den[:])
                    ot = finp.tile([128, C], f32, tag="ot")
                    # multiply on the otherwise-idle ACT engine:
                    # out = Copy(in * scale), scale broadcast per partition
                    nc.scalar.activation(
                        ot[:], ps[:, :C], mybir.ActivationFunctionType.Copy,
                        scale=inv[:],
                    )
                    nc.sync.dma_start(out=out_ext[w], in_=ot[:])
    nc.compile()
    return nc


def shard_inputs(feats, ids, cfg: Cfg):
    """Host: route rows to owner cores, bucket into 128-id windows."""
    import ml_dtypes

    ids = np.asarray(ids, dtype=np.int64).ravel()
    feats = np.asarray(feats, dtype=np.float32)
    owner = ids // cfg.rng
    local = (ids - owner * cfg.rng).astype(np.int32)
    order = np.argsort(owner, kind="stable")
    counts = np.bincount(owner, minlength=NCORES)
    offs = np.zeros(NCORES + 1, np.int64)
    np.cumsum(counts, out=offs[1:])
    feats_sorted = feats[order]
    local_sorted = local[order]

    in_maps = []
    iota = np.broadcast_to(
        np.arange(W, dtype=np.float32), (128, W)
    ).astype(ml_dtypes.bfloat16)
    for k in range(NCORES):
        fk = feats_sorted[offs[k] : offs[k + 1]]
        lk = local_sorted[offs[k] : offs[k + 1]]
        n_k = lk.shape[0]
        fa = np.zeros((cfg.s_tot, CW), np.float32)
        if n_k:
            sorder = np.argsort(lk, kind="stable")
            ls = lk[sorder]
            win = ls >> 7
            wcount = np.bincount(win, minlength=cfg.n_win)
            assert wcount.max() <= cfg.cap * 128, (
                f"window overflow {wcount.max()} > {cfg.cap * 128}"
            )
            wstart = np.zeros(cfg.n_win, np.int64)
            np.cumsum(wcount[:-1], out=wstart[1:])
            rank_in_win = np.arange(n_k) - wstart[win]
            dst = win * (cfg.cap * 128) + rank_in_win
            fa[dst, :C] = fk[sorder]
            fa[dst, C] = 1.0
            fa[dst, C + 1] = (ls & 127).astype(np.float32)
        arranged = np.ascontiguousarray(
            fa.reshape(cfg.s_slots, 128, CW).transpose(1, 0, 2)
        ).astype(ml_dtypes.bfloat16)
        in_maps.append({"feats": arranged, "iota": iota})
    return in_maps


def assemble_output(results, n_coarse, cfg: Cfg):
    out = np.empty((NCORES * cfg.rng, C), np.float32)
    for k in range(NCORES):
        out[k * cfg.rng : (k + 1) * cfg.rng] = results[k]["out"].reshape(
            cfg.rng, C
        )
    return out[:n_coarse]


def emulate_device(in_map, cfg: Cfg):
    feats = np.asarray(in_map["feats"], dtype=np.float32)  # [128, s_slots, CW]
    acc = np.zeros((cfg.n_win, W, CW - 1), np.float64)
    for s in range(cfg.s_slots):
        w = s // cfg.cap
        for p in range(128):
            row = feats[p, s]
            seg = int(row[CW - 1])
            acc[w, seg, :] += row[: CW - 1]
    den = np.maximum(acc[:, :, C], 1.0)[:, :, None]
    return {"out": (acc[:, :, :C] / den).astype(np.float32)}


def _install_axon_hooks_shim():
    """Provide antenv.axon_hooks + the ctypes NTFF hook if the image lacks it.

    Mirrors trn_agent_boot.trn_boot._ntff_profile_via_ctypes so that
    run_bass_kernel_spmd(trace=True) can profile under axon.
    """
    import contextlib
    import ctypes
    import types

    try:
        from antenv.axon_hooks import get_axon_ntff_profile_hook  # noqa: F401

        return
    except ImportError:
        pass
    import antenv

    mod = types.ModuleType("antenv.axon_hooks")
    state = {"h": None}
    mod.set_axon_ntff_profile_hook = lambda h: state.__setitem__("h", h)
    mod.get_axon_ntff_profile_hook = lambda: state["h"]
    antenv.axon_hooks = mod
    sys.modules["antenv.axon_hooks"] = mod

    so_path = "/opt/axon/libaxon_pjrt.so"
    if not os.path.exists(so_path):
        return
    lib = ctypes.CDLL(so_path)
    if not hasattr(lib, "axon_start_nrt_profile"):
        return
    lib.axon_start_nrt_profile.argtypes = [
        ctypes.POINTER(ctypes.c_int64),
        ctypes.c_size_t,
    ]
    lib.axon_start_nrt_profile.restype = ctypes.c_int64
    lib.axon_stop_nrt_profile.argtypes = [ctypes.c_char_p]
    lib.axon_stop_nrt_profile.restype = ctypes.c_int64

    @contextlib.contextmanager
    def _hook(output_dir, device_ids):
        import jax

        jax.devices()
        if device_ids:
            ids = (ctypes.c_int64 * len(device_ids))(*device_ids)
            rc = lib.axon_start_nrt_profile(ids, len(device_ids))
        else:
            rc = lib.axon_start_nrt_profile(None, 0)
        if rc != 0:
            raise RuntimeError(f"axon_start_nrt_profile rc={rc}")
        try:
            yield
        finally:
            n = lib.axon_stop_nrt_profile(str(output_dir).encode())
            print(f"profile: {n} file(s) written to {output_dir}", file=sys.stderr)

    state["h"] = _hook


def kernel(fine_feats, coarse_ids, num_coarse):
    global LAST_RESULT
    from concourse.bass_utils import run_bass_kernel_spmd

    cfg = CFG
    # adapt window capacity to the data (stays at the default for the
    # expected uniform-random ids; protects other distributions)
    ids64 = np.asarray(coarse_ids, dtype=np.int64).ravel()
    owner = ids64 // cfg.rng
    local = ids64 - owner * cfg.rng
    mx = 0
    for k in range(NCORES):
        lk = local[owner == k]
        if lk.size:
            mx = max(mx, int(np.bincount(lk >> 7, minlength=cfg.n_win).max()))
    need_cap = max(cfg.cap, -(-mx // 128))
    if need_cap != cfg.cap:
        cfg = Cfg(cap=need_cap)
    in_maps = shard_inputs(fine_feats, coarse_ids, cfg)
    key = ("full", cfg.cap)
    if key not in _nc_cache:
        _nc_cache[key] = build_nc(cfg)
    nc = _nc_cache[key]
    trace = bool(int(os.environ.get("KERNEL_TRACE", "0")))
    if trace:
        _install_axon_hooks_shim()
    res = run_bass_kernel_spmd(nc, in_maps, core_ids=list(range(NCORES)), trace=trace)
    LAST_RESULT = res
    return assemble_output(res.results, int(num_coarse), cfg)



# revision 2
# speedup vs baseline: 1.0961x; 1.0961x over previous
"""Sparse avg-pool (segment mean) for Trainium2, 8 NeuronCores — v3.

Changes vs v2 (which already prescales by 1/count on host and uses the
DVE 2x_1p fused one-hot):

- Dense token packing: a core's tokens, sorted by coarse id, fill slots of
  128 with NO per-window padding (v2 padded every 64-id window to 5 slots:
  12.8% waste).  The PE issues a fixed ~75ns LDWEIGHTS+MATMUL pair per slot
  regardless of width, so PE time scales with slot count alone.
- Overlapping groups: every 4 consecutive slots (512 tokens) form a group
  with a data-dependent base id (id of its first token).  512 sorted tokens
  span <= 80 ids with ~5 sigma margin (adaptive recompile if not), so the
  one-hot is [128 tokens, 80 segs] with group-relative ids.  Adjacent groups
  may cover overlapping id ranges; the host ADDS group outputs into the
  final array (partial sums are disjoint, so adding is exact).
- One-hot layout [p, seg, slot] with a fully materialized tiled iota
  [128, W, 32]: unit-stride last axis on all operands (2x_1p), inner runs of
  32 instead of v2's 5.
- bf16 device output (error gate is 2e-2, we run at ~2e-3): halves the
  output DMA.
"""
import os
import sys
from dataclasses import dataclass

sys.path.insert(0, "/opt/trn_rl_repo")

import numpy as np

NCORES = 8
C = 64
K = 2          # slots per group
SPG = 128 * K  # tokens per group


@dataclass(frozen=True)
class Cfg:
    n_coarse_pad: int = 250_880
    s_slots: int = 1984   # slots per core (992 groups of 2)
    w: int = 48           # one-hot segment width (max group id-span)
    spc0: int = 32        # starter chunk size (slots)
    spc: int = 64         # main chunk size (slots)
    drain: int = 8        # groups per PSUM tile / ACT copy
    dma_grp: int = 16     # groups per output DMA

    @property
    def rng(self):
        return self.n_coarse_pad // NCORES

    @property
    def n_grp(self):
        return self.s_slots // K

    @property
    def s_tot(self):
        return self.s_slots * 128


CFG = Cfg()
_nc_cache = {}
LAST_RESULT = None


def chunk_plan(cfg: Cfg):
    """Chunk sizes in slots: two small starter chunks to fill the pipeline
    fast, then large transfers to amortize per-DMA setup."""
    rem = cfg.s_slots - 2 * cfg.spc0
    assert rem % cfg.spc == 0
    return [cfg.spc0, cfg.spc0] + [cfg.spc] * (rem // cfg.spc)


def build_nc(cfg: Cfg):
    from concourse import bacc, mybir, tile

    bf16 = mybir.dt.bfloat16
    f32 = mybir.dt.float32
    nc = bacc.Bacc("TRN2", target_bir_lowering=False)
    plan = chunk_plan(cfg)
    feats_ext = nc.declare_dram_parameter(
        "feats", [128, cfg.s_slots, C], bf16, isOutput=False
    )
    ids_ext = nc.declare_dram_parameter(
        "ids", [128, cfg.s_slots], bf16, isOutput=False
    )
    iota_ext = nc.declare_dram_parameter(
        "iota", [128, cfg.w, K], bf16, isOutput=False
    )
    out_ext = nc.declare_dram_parameter(
        "out", [cfg.w, cfg.n_grp, C], bf16, isOutput=True
    )

    mspc = max(plan)
    mcg = mspc // K

    with tile.TileContext(nc) as tc:
        with (
            tc.tile_pool(name="stage", bufs=3) as stagep,
            tc.tile_pool(name="oh", bufs=3) as ohp,
            tc.tile_pool(name="psum", bufs=4, space="PSUM") as psump,
            tc.tile_pool(name="fin", bufs=4) as finp,
            tc.tile_pool(name="cst", bufs=1) as cstp,
        ):
            iota_t = cstp.tile([128, cfg.w, K], bf16)
            nc.sync.dma_start(out=iota_t[:], in_=iota_ext[:])
            ids_t = cstp.tile([128, cfg.s_slots], bf16)
            nc.sync.dma_start(out=ids_t[:], in_=ids_ext[:])

            col = 0
            g = 0
            for spc in plan:
                cg = spc // K
                s0 = col
                blk = stagep.tile([128, mspc, C], bf16, tag="src")
                nc.sync.dma_start(
                    out=blk[:, :spc, :],
                    in_=feats_ext[:, s0 : s0 + spc, :],
                )
                feats_v = blk[:, :spc, :]
                ids_v = ids_t[:, s0 : s0 + spc]
                oh = ohp.tile([128, mcg, cfg.w, K], bf16, tag="oh")
                nc.vector.tensor_tensor(
                    out=oh[:, :cg],
                    in0=ids_v.rearrange("p (g s) -> p g s", g=cg)
                    .unsqueeze(2)
                    .to_broadcast([128, cg, cfg.w, K]),
                    in1=iota_t[:]
                    .unsqueeze(1)
                    .to_broadcast([128, cg, cfg.w, K]),
                    op=mybir.AluOpType.is_equal,
                )
                for h0 in range(0, cg, cfg.dma_grp):
                    hn = min(cfg.dma_grp, cg - h0)
                    ot = finp.tile([cfg.w, cfg.dma_grp, C], bf16, tag="ot")
                    for d0 in range(h0, h0 + hn, cfg.drain):
                        dn = min(cfg.drain, h0 + hn - d0)
                        ps = psump.tile([cfg.w, cfg.drain, C], f32, tag="ps")
                        for gi in range(dn):
                            for s in range(K):
                                t = (d0 + gi) * K + s
                                nc.tensor.matmul(
                                    out=ps[:, gi, :],
                                    lhsT=oh[:, d0 + gi, :, s],
                                    rhs=feats_v[:, t, :],
                                    start=(s == 0),
                                    stop=(s == K - 1),
                                )
                        nc.scalar.activation(
                            ot[:, d0 - h0 : d0 - h0 + dn, :],
                            ps[:, :dn, :],
                            mybir.ActivationFunctionType.Copy,
                        )
                    nc.gpsimd.dma_start(
                        out=out_ext[:, g + h0 : g + h0 + hn, :],
                        in_=ot[:, :hn, :],
                    )
                col += spc
                g += cg
    nc.compile()
    return nc


def shard_inputs(feats, ids, cfg: Cfg):
    """Host: scale by 1/count, route to owner cores, sort by local id,
    pack densely, compute per-group base ids and group-relative ids."""
    import ml_dtypes

    ids = np.asarray(ids, dtype=np.int64).ravel()
    feats = np.asarray(feats, dtype=np.float32)
    cnt = np.bincount(ids, minlength=cfg.n_coarse_pad).astype(np.float32)
    scale = 1.0 / np.maximum(cnt, 1.0)
    feats = feats * scale[ids][:, None]

    owner = ids // cfg.rng
    local = (ids - owner * cfg.rng).astype(np.int64)
    order = np.argsort(owner, kind="stable")
    counts = np.bincount(owner, minlength=NCORES)
    offs = np.zeros(NCORES + 1, np.int64)
    np.cumsum(counts, out=offs[1:])
    feats_sorted = feats[order]
    local_sorted = local[order]

    iota = np.broadcast_to(
        np.arange(cfg.w, dtype=np.float32)[None, :, None], (128, cfg.w, K)
    ).astype(ml_dtypes.bfloat16)
    iota = np.ascontiguousarray(iota)

    in_maps = []
    bases_all = []
    max_span = 0
    for k in range(NCORES):
        fk = feats_sorted[offs[k] : offs[k + 1]]
        lk = local_sorted[offs[k] : offs[k + 1]]
        n_k = lk.shape[0]
        assert n_k <= cfg.s_tot, f"core {k}: {n_k} tokens > {cfg.s_tot}"
        sorder = np.argsort(lk, kind="stable")
        ls = lk[sorder]
        fs = fk[sorder]
        bases = np.zeros(cfg.n_grp, np.int64)
        ngrp_used = -(-n_k // SPG)
        bases[:ngrp_used] = ls[np.arange(ngrp_used) * SPG]
        rel = ls - bases[np.arange(n_k) // SPG]
        span = int(rel.max()) + 1 if n_k else 0
        max_span = max(max_span, span)
        fa = np.zeros((cfg.s_tot, C), np.float32)
        ia = np.full((cfg.s_tot,), -1.0, np.float32)
        fa[:n_k] = fs
        ia[:n_k] = rel.astype(np.float32)
        feats_arr = np.ascontiguousarray(
            fa.reshape(cfg.s_slots, 128, C).transpose(1, 0, 2)
        ).astype(ml_dtypes.bfloat16)
        ids_arr = np.ascontiguousarray(
            ia.reshape(cfg.s_slots, 128).T
        ).astype(ml_dtypes.bfloat16)
        in_maps.append({"feats": feats_arr, "ids": ids_arr, "iota": iota})
        bases_all.append(bases)
    return in_maps, bases_all, max_span


def assemble_output(results, bases_all, n_coarse, cfg: Cfg):
    out = np.empty((NCORES * cfg.rng, C), np.float32)
    for k in range(NCORES):
        dev = np.asarray(results[k]["out"], dtype=np.float32)  # [W, n_grp, C]
        acc = np.zeros((cfg.rng + cfg.w, C), np.float32)
        bases = bases_all[k]
        for g in range(cfg.n_grp):
            b = bases[g]
            acc[b : b + cfg.w] += dev[:, g, :]
        out[k * cfg.rng : (k + 1) * cfg.rng] = acc[: cfg.rng]
    return out[:n_coarse]


def _install_axon_hooks_shim():
    """Provide antenv.axon_hooks + the ctypes NTFF hook if the image lacks it."""
    import contextlib
    import ctypes
    import types

    try:
        from antenv.axon_hooks import get_axon_ntff_profile_hook  # noqa: F401

        return
    except ImportError:
        pass
    import antenv

    mod = types.ModuleType("antenv.axon_hooks")
    state = {"h": None}
    mod.set_axon_ntff_profile_hook = lambda h: state.__setitem__("h", h)
    mod.get_axon_ntff_profile_hook = lambda: state["h"]
    antenv.axon_hooks = mod
    sys.modules["antenv.axon_hooks"] = mod

    so_path = "/opt/axon/libaxon_pjrt.so"
    if not os.path.exists(so_path):
        return
    lib = ctypes.CDLL(so_path)
    if not hasattr(lib, "axon_start_nrt_profile"):
        return
    lib.axon_start_nrt_profile.argtypes = [
        ctypes.POINTER(ctypes.c_int64),
        ctypes.c_size_t,
    ]
    lib.axon_start_nrt_profile.restype = ctypes.c_int64
    lib.axon_stop_nrt_profile.argtypes = [ctypes.c_char_p]
    lib.axon_stop_nrt_profile.restype = ctypes.c_int64

    @contextlib.contextmanager
    def _hook(output_dir, device_ids):
        import jax

        jax.devices()
        if device_ids:
            ids = (ctypes.c_int64 * len(device_ids))(*device_ids)
            rc = lib.axon_start_nrt_profile(ids, len(device_ids))
        else:
            rc = lib.axon_start_nrt_profile(None, 0)
        if rc != 0:
            raise RuntimeError(f"axon_start_nrt_profile rc={rc}")
        try:
            yield
        finally:
            n = lib.axon_stop_nrt_profile(str(output_dir).encode())
            print(f"profile: {n} file(s) written to {output_dir}", file=sys.stderr)

    state["h"] = _hook


def kernel(fine_feats, coarse_ids, num_coarse):
    global LAST_RESULT
    from concourse.bass_utils import run_bass_kernel_spmd

    cfg = CFG
    # guard: if the data needs more slots or wider groups than compiled,
    # escalate the config (never triggers for uniform-random ids)
    ids64 = np.asarray(coarse_ids, dtype=np.int64).ravel()
    owner = ids64 // cfg.rng
    nmax = int(np.bincount(owner, minlength=NCORES).max())
    s_need = max(cfg.s_slots, -(-nmax // 128))
    if s_need > cfg.s_slots:
        rem = -(-(s_need - 2 * cfg.spc0) // cfg.spc) * cfg.spc
        cfg = Cfg(s_slots=2 * cfg.spc0 + rem)
    while True:
        in_maps, bases_all, max_span = shard_inputs(fine_feats, coarse_ids, cfg)
        if max_span <= cfg.w:
            break
        w_need = min(128, -(-max_span // 16) * 16)
        assert w_need <= 128, f"group span {max_span} > 128"
        cfg = Cfg(s_slots=cfg.s_slots, w=w_need)
    key = ("v3", cfg.s_slots, cfg.w)
    if key not in _nc_cache:
        _nc_cache[key] = build_nc(cfg)
    nc = _nc_cache[key]
    trace = bool(int(os.environ.get("KERNEL_TRACE", "0")))
    if trace:
        _install_axon_hooks_shim()
    res = run_bass_kernel_spmd(nc, in_maps, core_ids=list(range(NCORES)), trace=trace)
    LAST_RESULT = res
    return assemble_output(res.results, bases_all, int(num_coarse), cfg)
